# revision 9
# baseline (speedup 1.0000x reference)
"""Graph Wavelet NN (2-layer) Trainium2 kernel, 8-core row-parallel, v3.

Math per layer: out = (wavelets * f) @ (wavelets_inv @ (x @ W)); the filter is
folded into a row-scale of the small spectral tensor s.

Design (v1 ~477us, v2 ~550us measured):
- t1 = x @ W1 computed FULLY REPLICATED per core (27us of PE work with no
  cross-core deps) so layer 1 needs no input exchange and the one-time ~47us
  collective-stream init (absorbed by a dummy AllGather issued at t=0) never
  stalls compute.
- winvT SBUF-RESIDENT (16MB, filled during t1/s1, reused by s2); wT streamed
  twice (o1, o2) so no phase window stacks two 16MB streams.
- s1 computed in row-quarter passes, each quarter stored + AllGather'd
  immediately (pipelines the s1 exchange against s1 itself).  t2 likewise
  quartered.  s2/o1/o2 are slot-pipelined single passes consuming gathered
  granules in arrival order.
- EVERYTHING IS STATIC: one pid-INDEPENDENT permutation (quarter-major,
  rank-major 256-row granules) is baked into the host-side layouts of xT,
  winvT and wT, and consumers read every granule (own rank included) from the
  gathered buffers at static offsets.  v2 used pid-dependent dynamic-offset
  DMAs whose register setup swamped the scalar queue (251us busy) and paced
  the whole kernel.
- Collectives alone on gpsimd; winv/wT streams on sync; small loads/stores/
  gather reads on scalar.  bf16 matmuls, fp32 PSUM accumulation.
"""

import sys

if "/opt/trn_rl_repo" not in sys.path:
    sys.path.insert(0, "/opt/trn_rl_repo")

import numpy as np
import ml_dtypes

import concourse.bass as bass
import concourse.mybir as mybir
import concourse.tile as tile
from concourse import bacc, bass_utils

N = 8192
F = 512
C = 256
NCORES = 8
R = N // NCORES          # 1024 rows per core
Q = R // 4               # 256-row quarter granule
NSLOT = N // 128         # 64 contraction slots of 128 rows

F32 = mybir.dt.float32
BF16 = mybir.dt.bfloat16
NP_BF16 = ml_dtypes.bfloat16


def build_kernel(sim_single_core=False):
    nc = bacc.Bacc(
        "TRN2",
        target_bir_lowering=False,
        debug=False,
        num_devices=1 if sim_single_core else NCORES,
    )

    xT = nc.dram_tensor("xT", [F, N], BF16, kind="ExternalInput")
    w1 = nc.dram_tensor("w1", [F, C], BF16, kind="ExternalInput")
    w2 = nc.dram_tensor("w2", [C, C], BF16, kind="ExternalInput")
    winvT = nc.dram_tensor("winvT", [N, R], BF16, kind="ExternalInput")
    wT = nc.dram_tensor("wT", [N, R], BF16, kind="ExternalInput")
    f1 = nc.dram_tensor("f1", [R], F32, kind="ExternalInput")
    f2 = nc.dram_tensor("f2", [R], F32, kind="ExternalInput")
    outT = nc.dram_tensor("outT", [C, R], F32, kind="ExternalOutput")

    rg = [list(range(NCORES))]

    with tile.TileContext(nc) as tc:
        with (
            tc.tile_pool(name="dram", bufs=1, space="DRAM") as dram,
            tc.tile_pool(name="const", bufs=1) as const,
            tc.tile_pool(name="wtp", bufs=2) as wtp,
            tc.tile_pool(name="xqp", bufs=2) as xqp,
            tc.tile_pool(name="tsp", bufs=4) as tsp,
            tc.tile_pool(name="psq", bufs=2, space="PSUM") as psq,
            tc.tile_pool(name="psO", bufs=2, space="PSUM") as psO,
        ):
            # ---- DRAM exchange buffers (quarter granules) ----
            def mk_pair(nm):
                ins, outs = [], []
                for b in range(4):
                    ins.append(dram.tile([Q, C], BF16, name=f"{nm}{b}_d"))
                    outs.append(
                        dram.tile(
                            [NCORES * Q, C], BF16,
                            addr_space="Local" if sim_single_core else "Shared",
                            name=f"{nm}{b}g_d",
                        )
                    )
                return ins, outs

            s1q_d, s1g_d = mk_pair("s1")
            t2q_d, t2g_d = mk_pair("t2")
            s2q_d, s2g_d = mk_pair("s2")

            # dummy collective: starts the ncfw stream init at t~0 so the
            # first real AllGather doesn't eat the ~47us one-time cost.
            dum_i = dram.tile([Q, 1], BF16, name="dum_i")
            dum_o = dram.tile(
                [NCORES * Q, 1], BF16,
                addr_space="Local" if sim_single_core else "Shared",
                name="dum_o",
            )

            # ---- persistent SBUF ----
            winv_sb = const.tile([128, NSLOT, R], BF16)     # 128KB/part
            t1f_sb = const.tile([128, NSLOT, C], BF16)      # full t1, pi order
            t_sb2 = const.tile([128, 8, C], BF16)           # local t2 rows
            s_sb1 = const.tile([128, 8, C], BF16)           # local s1 rows
            s_sb2 = const.tile([128, 8, C], BF16)           # local s2 rows
            h1T_sb = const.tile([128, C // 128, R], BF16)   # relu(o1).T
            w1_sb = const.tile([128, F // 128, C], BF16)
            w2_sb = const.tile([128, C // 128, C], BF16)
            f1_sb = const.tile([128, 8], F32)
            f2_sb = const.tile([128, 8], F32)

            def all_gather(in_d, out_d):
                if sim_single_core:
                    rows = in_d.shape[0]
                    for rr in range(NCORES):
                        nc.sync.dma_start(
                            out=out_d[rr * rows:(rr + 1) * rows, :], in_=in_d[:, :]
                        )
                else:
                    nc.gpsimd.collective_compute(
                        "AllGather",
                        mybir.AluOpType.bypass,
                        replica_groups=rg,
                        ins=[in_d.opt()],
                        outs=[out_d.opt()],
                    )

            all_gather(dum_i, dum_o)

            nc.scalar.dma_start(
                out=w1_sb[:], in_=w1.ap().rearrange("(kc p) n -> p kc n", p=128)
            )
            nc.scalar.dma_start(
                out=w2_sb[:], in_=w2.ap().rearrange("(kc p) n -> p kc n", p=128)
            )
            nc.scalar.dma_start(
                out=f1_sb[:], in_=f1.ap().rearrange("(mt p) -> p mt", p=128)
            )
            nc.scalar.dma_start(
                out=f2_sb[:], in_=f2.ap().rearrange("(mt p) -> p mt", p=128)
            )

            # winv fill: q-major so s1's quarter-0 pass streams just-in-time.
            for q in range(4):
                for g in range(8):
                    nc.sync.dma_start(
                        out=winv_sb[:, g * 8:(g + 1) * 8, q * Q:(q + 1) * Q],
                        in_=winvT.ap()[
                            g * 1024:(g + 1) * 1024, q * Q:(q + 1) * Q
                        ].rearrange("(kc p) m -> p kc m", p=128),
                    )

            # ======= t1 = x @ W1, fully replicated, staged in pi order =======
            # xT is host-permuted; granule gi = 256 pi-columns = slots 2gi,2gi+1
            for gi in range(32):
                xq = xqp.tile([128, 4, C], BF16, tag="xq", name=f"xq{gi}")
                nc.scalar.dma_start(
                    out=xq[:],
                    in_=xT.ap()[:, gi * Q:(gi + 1) * Q].rearrange(
                        "(kc p) m -> p kc m", p=128
                    ),
                )
                pt = psq.tile([128, 2, C], F32, tag="ps", name=f"pt1_{gi}")
                for j in range(2):
                    for kc in range(4):
                        nc.tensor.matmul(
                            pt[:, j, :],
                            xq[:, kc, j * 128:(j + 1) * 128],
                            w1_sb[:, kc, :],
                            start=(j == 0 and kc == 0),
                            stop=(kc == 3),
                            skip_group_check=True,
                        )
                    nc.vector.tensor_copy(t1f_sb[:, 2 * gi + j, :], pt[:, j, :])

            # ======= s1 = Winv @ t1 (all SBUF), row-quarter passes + AG =======
            for q in range(4):
                ps = psq.tile([128, 2, C], F32, tag="ps", name=f"ps1_{q}")
                for p in range(NSLOT):
                    for j in range(2):
                        nc.tensor.matmul(
                            ps[:, j, :],
                            winv_sb[:, p, q * Q + j * 128:q * Q + (j + 1) * 128],
                            t1f_sb[:, p, :],
                            start=(p == 0 and j == 0),
                            stop=(p == NSLOT - 1),
                            skip_group_check=True,
                        )
                for j in range(2):
                    nc.vector.tensor_scalar_mul(
                        s_sb1[:, 2 * q + j, :],
                        ps[:, j, :],
                        f1_sb[:, 2 * q + j:2 * q + j + 1],
                    )
                nc.scalar.dma_start(
                    out=s1q_d[q][:, :].rearrange("(k p) n -> p k n", p=128),
                    in_=s_sb1[:, 2 * q:2 * q + 2, :],
                )
                all_gather(s1q_d[q], s1g_d[q])

            # ---- o phase: out_loc = (w[rows]*f) @ s_full, slot-pipelined ----
            # consumes gathered granules (own rank included) in static
            # (quarter, rank) order == pi order == AG arrival order.
            def o_phase(sg_d, drain_cb, name):
                po = [
                    psO.tile([128, R], F32, tag="po", name=f"po_{name}{ch}")
                    for ch in range(2)
                ]
                wt_tiles = {}

                def load_wt(g):
                    t = wtp.tile([128, 4, R], BF16, tag="wt", name=f"wt_{name}{g}")
                    nc.sync.dma_start(
                        out=t[:],
                        in_=wT.ap()[g * 512:(g + 1) * 512, :].rearrange(
                            "(kc p) m -> p kc m", p=128
                        ),
                    )
                    wt_tiles[g] = t

                def mm(p, lhsT_of):
                    g = p // 4
                    for ch in range(2):
                        for mh in range(2):
                            nc.tensor.matmul(
                                po[ch][:, mh * 512:(mh + 1) * 512],
                                lhsT_of(ch),
                                wt_tiles[g][:, p % 4, mh * 512:(mh + 1) * 512],
                                start=(p == 0),
                                stop=(p == NSLOT - 1),
                                skip_group_check=True,
                            )

                load_wt(0)
                load_wt(1)
                for qq in range(4):
                    for rk in range(NCORES):
                        r = qq * 8 + rk
                        sgt = tsp.tile(
                            [128, 2, C], BF16, tag="ts", name=f"so_{name}_{r}"
                        )
                        nc.scalar.dma_start(
                            out=sgt[:],
                            in_=sg_d[qq][rk * Q:(rk + 1) * Q, :].rearrange(
                                "(k p) n -> p k n", p=128
                            ),
                        )
                        if r % 2 == 0 and 2 + r // 2 < 16:
                            load_wt(2 + r // 2)
                        for k in range(2):
                            mm(
                                2 * r + k,
                                lambda ch, _t=sgt, _k=k: _t[
                                    :, _k, ch * 128:(ch + 1) * 128
                                ],
                            )
                for ch in range(2):
                    drain_cb(ch, po[ch])

            # ================= layer 1 out =================
            def relu_drain(ch, po):
                for mh in range(2):
                    nc.vector.tensor_scalar_max(
                        h1T_sb[:, ch, mh * 512:(mh + 1) * 512],
                        po[:, mh * 512:(mh + 1) * 512],
                        0.0,
                    )

            o_phase(s1g_d, relu_drain, "o1")

            # ======= t2 = relu(o1) @ W2 (local rows), quartered + AG =======
            for q in range(4):
                pt = psq.tile([128, 2, C], F32, tag="ps", name=f"pt2_{q}")
                for j in range(2):
                    mt = 2 * q + j
                    for kc in range(2):
                        nc.tensor.matmul(
                            pt[:, j, :],
                            h1T_sb[:, kc, mt * 128:(mt + 1) * 128],
                            w2_sb[:, kc, :],
                            start=(j == 0 and kc == 0),
                            stop=(kc == 1),
                            skip_group_check=True,
                        )
                    nc.vector.tensor_copy(t_sb2[:, mt, :], pt[:, j, :])
                nc.scalar.dma_start(
                    out=t2q_d[q][:, :].rearrange("(k p) n -> p k n", p=128),
                    in_=t_sb2[:, 2 * q:2 * q + 2, :],
                )
                all_gather(t2q_d[q], t2g_d[q])

            # ======= s2 = Winv @ t2_full, slot-pipelined single pass =======
            ps2 = [
                psq.tile([128, 2, 2, C], F32, tag="ps", name=f"ps2_{i}")
                for i in range(2)
            ]

            def s2_mm(p, rhs):
                for mt in range(8):
                    nc.tensor.matmul(
                        ps2[mt // 4][:, (mt % 4) // 2, (mt % 4) % 2, :],
                        winv_sb[:, p, mt * 128:(mt + 1) * 128],
                        rhs,
                        start=(p == 0 and mt % 2 == 0),
                        stop=(p == NSLOT - 1),
                        skip_group_check=True,
                    )

            for qq in range(4):
                for rk in range(NCORES):
                    r = qq * 8 + rk
                    tsg = tsp.tile([128, 2, C], BF16, tag="ts", name=f"ts2_{r}")
                    nc.scalar.dma_start(
                        out=tsg[:],
                        in_=t2g_d[qq][rk * Q:(rk + 1) * Q, :].rearrange(
                            "(k p) n -> p k n", p=128
                        ),
                    )
                    for k in range(2):
                        s2_mm(2 * r + k, tsg[:, k, :])

            for q in range(4):
                for j in range(2):
                    mt = 2 * q + j
                    nc.vector.tensor_scalar_mul(
                        s_sb2[:, mt, :],
                        ps2[mt // 4][:, (mt % 4) // 2, (mt % 4) % 2, :],
                        f2_sb[:, mt:mt + 1],
                    )
                nc.scalar.dma_start(
                    out=s2q_d[q][:, :].rearrange("(k p) n -> p k n", p=128),
                    in_=s_sb2[:, 2 * q:2 * q + 2, :],
                )
                all_gather(s2q_d[q], s2g_d[q])

            # ================= layer 2 out =================
            # out_st reuses a "wt" slot; allocated lazily AFTER o2's last wT
            # tile so the ring rotation never makes a wT load wait on the
            # final output stores.
            _oh = {}

            def out_drain(ch, po):
                if "t" not in _oh:
                    _oh["t"] = wtp.tile([128, 2, R], F32, tag="wt", name="out_st")
                out_st = _oh["t"]
                for mh in range(2):
                    nc.vector.tensor_copy(
                        out_st[:, ch, mh * 512:(mh + 1) * 512],
                        po[:, mh * 512:(mh + 1) * 512],
                    )
                    nc.scalar.dma_start(
                        out=outT.ap()[
                            ch * 128:(ch + 1) * 128, mh * 512:(mh + 1) * 512
                        ],
                        in_=out_st[:, ch, mh * 512:(mh + 1) * 512],
                    )

            o_phase(s2g_d, out_drain, "o2")

    nc.compile()
    return nc


_NC_CACHE = {}


def _get_nc():
    if "nc" not in _NC_CACHE:
        _NC_CACHE["nc"] = build_kernel()
    return _NC_CACHE["nc"]


# global pid-independent contraction order: quarter-major, rank-major
_PERM = np.concatenate(
    [
        np.arange(rk * R + q * Q, rk * R + q * Q + Q)
        for q in range(4)
        for rk in range(NCORES)
    ]
)


def make_in_maps(input, wavelets, wavelets_inv, W1, W2, filter1, filter2):
    input = np.asarray(input, np.float32)
    wavelets = np.asarray(wavelets, np.float32)
    wavelets_inv = np.asarray(wavelets_inv, np.float32)
    W1b = np.ascontiguousarray(np.asarray(W1, np.float32)).astype(NP_BF16)
    W2b = np.ascontiguousarray(np.asarray(W2, np.float32)).astype(NP_BF16)
    filter1 = np.asarray(filter1, np.float32)
    filter2 = np.asarray(filter2, np.float32)

    xT_pi = np.ascontiguousarray(
        input.T[:, _PERM].astype(NP_BF16)
    )  # [F, N], pi cols; identical on every core
    in_maps = []
    for i in range(NCORES):
        r0, r1 = i * R, (i + 1) * R
        winvT_i = np.ascontiguousarray(wavelets_inv[r0:r1].T[_PERM]).astype(
            NP_BF16
        )
        wT_i = np.ascontiguousarray(wavelets[r0:r1].T[_PERM]).astype(NP_BF16)
        in_maps.append(
            {
                "xT": xT_pi,
                "w1": W1b,
                "w2": W2b,
                "winvT": winvT_i,
                "wT": wT_i,
                "f1": np.ascontiguousarray(filter1[r0:r1]),
                "f2": np.ascontiguousarray(filter2[r0:r1]),
            }
        )
    return in_maps


def run(in_maps, trace=False, **kw):
    nc = _get_nc()
    return bass_utils.run_bass_kernel_spmd(
        nc, in_maps, core_ids=list(range(NCORES)), trace=trace, **kw
    )


def kernel(input, wavelets, wavelets_inv, W1, W2, filter1, filter2):
    in_maps = make_in_maps(
        input, wavelets, wavelets_inv, W1, W2, filter1, filter2
    )
    res = run(in_maps)
    out = np.empty((N, C), np.float32)
    for i in range(NCORES):
        out[i * R:(i + 1) * R, :] = res.results[i]["outT"].T
    return out


# revision 11
# speedup vs baseline: 1.0726x; 1.0726x over previous
"""Graph Wavelet NN (2-layer) Trainium2 kernel, 8-core row-parallel, v3.

Math per layer: out = (wavelets * f) @ (wavelets_inv @ (x @ W)); the filter is
folded into a row-scale of the small spectral tensor s.

Design (v1 ~477us, v2 ~550us measured):
- t1 = x @ W1 computed FULLY REPLICATED per core (27us of PE work with no
  cross-core deps) so layer 1 needs no input exchange and the one-time ~47us
  collective-stream init (absorbed by a dummy AllGather issued at t=0) never
  stalls compute.
- winvT SBUF-RESIDENT (16MB, filled during t1/s1, reused by s2); wT streamed
  twice (o1, o2) so no phase window stacks two 16MB streams.
- s1 computed in row-quarter passes, each quarter stored + AllGather'd
  immediately (pipelines the s1 exchange against s1 itself).  t2 likewise
  quartered.  s2/o1/o2 are slot-pipelined single passes consuming gathered
  granules in arrival order.
- EVERYTHING IS STATIC: one pid-INDEPENDENT permutation (quarter-major,
  rank-major 256-row granules) is baked into the host-side layouts of xT,
  winvT and wT, and consumers read every granule (own rank included) from the
  gathered buffers at static offsets.  v2 used pid-dependent dynamic-offset
  DMAs whose register setup swamped the scalar queue (251us busy) and paced
  the whole kernel.
- Collectives alone on gpsimd; winv/wT streams on sync; small loads/stores/
  gather reads on scalar.  bf16 matmuls, fp32 PSUM accumulation.
"""

import sys

if "/opt/trn_rl_repo" not in sys.path:
    sys.path.insert(0, "/opt/trn_rl_repo")

import numpy as np
import ml_dtypes

import concourse.bass as bass
import concourse.mybir as mybir
import concourse.tile as tile
from concourse import bacc, bass_utils

N = 8192
F = 512
C = 256
NCORES = 8
R = N // NCORES          # 1024 rows per core
Q = R // 4               # 256-row quarter granule
NSLOT = N // 128         # 64 contraction slots of 128 rows

F32 = mybir.dt.float32
BF16 = mybir.dt.bfloat16
NP_BF16 = ml_dtypes.bfloat16


def build_kernel(sim_single_core=False):
    nc = bacc.Bacc(
        "TRN2",
        target_bir_lowering=False,
        debug=False,
        num_devices=1 if sim_single_core else NCORES,
    )

    xT = nc.dram_tensor("xT", [32 * F, Q], BF16, kind="ExternalInput")
    w1 = nc.dram_tensor("w1", [F, C], BF16, kind="ExternalInput")
    w2 = nc.dram_tensor("w2", [C, C], BF16, kind="ExternalInput")
    winvT = nc.dram_tensor("winvT", [4 * N, Q], BF16, kind="ExternalInput")
    wT = nc.dram_tensor("wT", [N, R], BF16, kind="ExternalInput")
    f1 = nc.dram_tensor("f1", [R], F32, kind="ExternalInput")
    f2 = nc.dram_tensor("f2", [R], F32, kind="ExternalInput")
    outT = nc.dram_tensor("outT", [C, R], F32, kind="ExternalOutput")

    rg = [list(range(NCORES))]

    with tile.TileContext(nc) as tc:
        with (
            tc.tile_pool(name="dram", bufs=1, space="DRAM") as dram,
            tc.tile_pool(name="const", bufs=1) as const,
            tc.tile_pool(name="wtp", bufs=2) as wtp,
            tc.tile_pool(name="xqp", bufs=2) as xqp,
            tc.tile_pool(name="tsp", bufs=4) as tsp,
            tc.tile_pool(name="psq", bufs=2, space="PSUM") as psq,
            tc.tile_pool(name="psO", bufs=2, space="PSUM") as psO,
        ):
            # ---- DRAM exchange buffers ----
            def mk_pair(nm, rows, nblk):
                ins, outs = [], []
                for b in range(nblk):
                    ins.append(dram.tile([rows, C], BF16, name=f"{nm}{b}_d"))
                    outs.append(
                        dram.tile(
                            [NCORES * rows, C], BF16,
                            addr_space="Local" if sim_single_core else "Shared",
                            name=f"{nm}{b}g_d",
                        )
                    )
                return ins, outs

            # E2 quarters (pipeline under s1); E3/E4 halves (fewer AG floors)
            s1q_d, s1g_d = mk_pair("s1", Q, 4)
            t2h_d, t2g_d = mk_pair("t2", 2 * Q, 2)
            s2h_d, s2g_d = mk_pair("s2", 2 * Q, 2)

            # dummy collective: starts the ncfw stream init at t~0 so the
            # first real AllGather doesn't eat the ~47us one-time cost.
            dum_i = dram.tile([Q, 1], BF16, name="dum_i")
            dum_o = dram.tile(
                [NCORES * Q, 1], BF16,
                addr_space="Local" if sim_single_core else "Shared",
                name="dum_o",
            )

            # ---- persistent SBUF ----
            winv_sb = const.tile([128, 4, NSLOT, Q], BF16)  # 128KB/part, q-major
            t1f_sb = const.tile([128, NSLOT, C], BF16)      # full t1, pi order
            t_sb2 = const.tile([128, 8, C], BF16)           # local t2 rows
            s_sb1 = const.tile([128, 8, C], BF16)           # local s1 rows
            s_sb2 = const.tile([128, 8, C], BF16)           # local s2 rows
            h1T_sb = const.tile([128, C // 128, R], BF16)   # relu(o1).T
            w1_sb = const.tile([128, F // 128, C], BF16)
            w2_sb = const.tile([128, C // 128, C], BF16)
            f1_sb = const.tile([128, 8], F32)
            f2_sb = const.tile([128, 8], F32)

            def all_gather(in_d, out_d):
                if sim_single_core:
                    rows = in_d.shape[0]
                    for rr in range(NCORES):
                        nc.sync.dma_start(
                            out=out_d[rr * rows:(rr + 1) * rows, :], in_=in_d[:, :]
                        )
                else:
                    nc.gpsimd.collective_compute(
                        "AllGather",
                        mybir.AluOpType.bypass,
                        replica_groups=rg,
                        ins=[in_d.opt()],
                        outs=[out_d.opt()],
                    )

            all_gather(dum_i, dum_o)

            nc.scalar.dma_start(
                out=w1_sb[:], in_=w1.ap().rearrange("(kc p) n -> p kc n", p=128)
            )
            nc.scalar.dma_start(
                out=w2_sb[:], in_=w2.ap().rearrange("(kc p) n -> p kc n", p=128)
            )
            nc.scalar.dma_start(
                out=f1_sb[:], in_=f1.ap().rearrange("(mt p) -> p mt", p=128)
            )
            nc.scalar.dma_start(
                out=f2_sb[:], in_=f2.ap().rearrange("(mt p) -> p mt", p=128)
            )

            # winv fill: q-major so s1's quarter-0 pass streams just-in-time.
            for q in range(4):
                for g in range(8):
                    nc.sync.dma_start(
                        out=winv_sb[:, q, g * 8:(g + 1) * 8, :],
                        in_=winvT.ap()[
                            q * N + g * 1024:q * N + (g + 1) * 1024, :
                        ].rearrange("(kc p) m -> p kc m", p=128),
                    )

            # ======= t1 = x @ W1, fully replicated, staged in pi order =======
            # xT is host-permuted; granule gi = 256 pi-columns = slots 2gi,2gi+1
            for gi in range(32):
                xq = xqp.tile([128, 4, C], BF16, tag="xq", name=f"xq{gi}")
                nc.scalar.dma_start(
                    out=xq[:],
                    in_=xT.ap()[gi * F:(gi + 1) * F, :].rearrange(
                        "(kc p) m -> p kc m", p=128
                    ),
                )
                pt = psq.tile([128, 2, C], F32, tag="ps", name=f"pt1_{gi}")
                for j in range(2):
                    for kc in range(4):
                        nc.tensor.matmul(
                            pt[:, j, :],
                            xq[:, kc, j * 128:(j + 1) * 128],
                            w1_sb[:, kc, :],
                            start=(j == 0 and kc == 0),
                            stop=(kc == 3),
                            skip_group_check=True,
                        )
                    nc.vector.tensor_copy(t1f_sb[:, 2 * gi + j, :], pt[:, j, :])

            # ======= s1 = Winv @ t1 (all SBUF), row-quarter passes + AG =======
            for q in range(4):
                ps = psq.tile([128, 2, C], F32, tag="ps", name=f"ps1_{q}")
                for p in range(NSLOT):
                    for j in range(2):
                        nc.tensor.matmul(
                            ps[:, j, :],
                            winv_sb[:, q, p, j * 128:(j + 1) * 128],
                            t1f_sb[:, p, :],
                            start=(p == 0 and j == 0),
                            stop=(p == NSLOT - 1),
                            skip_group_check=True,
                        )
                for j in range(2):
                    nc.vector.tensor_scalar_mul(
                        s_sb1[:, 2 * q + j, :],
                        ps[:, j, :],
                        f1_sb[:, 2 * q + j:2 * q + j + 1],
                    )
                nc.scalar.dma_start(
                    out=s1q_d[q][:, :].rearrange("(k p) n -> p k n", p=128),
                    in_=s_sb1[:, 2 * q:2 * q + 2, :],
                )
                all_gather(s1q_d[q], s1g_d[q])

            # ---- o phase: out_loc = (w[rows]*f) @ s_full, slot-pipelined ----
            # consumes gathered granules (own rank included) in static
            # (quarter, rank) order == pi order == AG arrival order.
            def o_phase(gran, drain_cb, name):
                po = [
                    psO.tile([128, R], F32, tag="po", name=f"po_{name}{ch}")
                    for ch in range(2)
                ]
                wt_tiles = {}

                def load_wt(g):
                    t = wtp.tile([128, 4, R], BF16, tag="wt", name=f"wt_{name}{g}")
                    nc.sync.dma_start(
                        out=t[:],
                        in_=wT.ap()[g * 512:(g + 1) * 512, :].rearrange(
                            "(kc p) m -> p kc m", p=128
                        ),
                    )
                    wt_tiles[g] = t

                def mm(p, lhsT_of):
                    g = p // 4
                    for ch in range(2):
                        for mh in range(2):
                            nc.tensor.matmul(
                                po[ch][:, mh * 512:(mh + 1) * 512],
                                lhsT_of(ch),
                                wt_tiles[g][:, p % 4, mh * 512:(mh + 1) * 512],
                                start=(p == 0),
                                stop=(p == NSLOT - 1),
                                skip_group_check=True,
                            )

                load_wt(0)
                load_wt(1)
                for qq in range(4):
                    for rk in range(NCORES):
                        r = qq * 8 + rk
                        sgt = tsp.tile(
                            [128, 2, C], BF16, tag="ts", name=f"so_{name}_{r}"
                        )
                        nc.scalar.dma_start(
                            out=sgt[:],
                            in_=gran(qq, rk).rearrange("(k p) n -> p k n", p=128),
                        )
                        if r % 2 == 0 and 2 + r // 2 < 16:
                            load_wt(2 + r // 2)
                        for k in range(2):
                            mm(
                                2 * r + k,
                                lambda ch, _t=sgt, _k=k: _t[
                                    :, _k, ch * 128:(ch + 1) * 128
                                ],
                            )
                for ch in range(2):
                    drain_cb(ch, po[ch])

            # ================= layer 1 out =================
            def relu_drain(ch, po):
                for mh in range(2):
                    nc.vector.tensor_scalar_max(
                        h1T_sb[:, ch, mh * 512:(mh + 1) * 512],
                        po[:, mh * 512:(mh + 1) * 512],
                        0.0,
                    )

            o_phase(
                lambda qq, rk: s1g_d[qq][rk * Q:(rk + 1) * Q, :],
                relu_drain,
                "o1",
            )

            # ======= t2 = relu(o1) @ W2 (local rows), AG per half =======
            for h in range(2):
                for qq in range(2):
                    q = 2 * h + qq
                    pt = psq.tile([128, 2, C], F32, tag="ps", name=f"pt2_{q}")
                    for j in range(2):
                        mt = 2 * q + j
                        for kc in range(2):
                            nc.tensor.matmul(
                                pt[:, j, :],
                                h1T_sb[:, kc, mt * 128:(mt + 1) * 128],
                                w2_sb[:, kc, :],
                                start=(j == 0 and kc == 0),
                                stop=(kc == 1),
                                skip_group_check=True,
                            )
                        nc.vector.tensor_copy(t_sb2[:, mt, :], pt[:, j, :])
                nc.scalar.dma_start(
                    out=t2h_d[h][:, :].rearrange("(k p) n -> p k n", p=128),
                    in_=t_sb2[:, 4 * h:4 * h + 4, :],
                )
                all_gather(t2h_d[h], t2g_d[h])

            # ======= s2 = Winv @ t2_full, slot-pipelined single pass =======
            ps2 = [
                psq.tile([128, 2, 2, C], F32, tag="ps", name=f"ps2_{i}")
                for i in range(2)
            ]

            def s2_mm(p, rhs):
                for mt in range(8):
                    nc.tensor.matmul(
                        ps2[mt // 4][:, (mt % 4) // 2, (mt % 4) % 2, :],
                        winv_sb[:, mt // 2, p, (mt % 2) * 128:(mt % 2 + 1) * 128],
                        rhs,
                        start=(p == 0 and mt % 2 == 0),
                        stop=(p == NSLOT - 1),
                        skip_group_check=True,
                    )

            for qq in range(4):
                for rk in range(NCORES):
                    r = qq * 8 + rk
                    tsg = tsp.tile([128, 2, C], BF16, tag="ts", name=f"ts2_{r}")
                    nc.scalar.dma_start(
                        out=tsg[:],
                        in_=t2g_d[qq // 2][
                            rk * 2 * Q + (qq % 2) * Q:
                            rk * 2 * Q + (qq % 2 + 1) * Q, :
                        ].rearrange("(k p) n -> p k n", p=128),
                    )
                    for k in range(2):
                        s2_mm(2 * r + k, tsg[:, k, :])

            for h in range(2):
                for j in range(4):
                    mt = 4 * h + j
                    nc.vector.tensor_scalar_mul(
                        s_sb2[:, mt, :],
                        ps2[mt // 4][:, (mt % 4) // 2, (mt % 4) % 2, :],
                        f2_sb[:, mt:mt + 1],
                    )
                nc.scalar.dma_start(
                    out=s2h_d[h][:, :].rearrange("(k p) n -> p k n", p=128),
                    in_=s_sb2[:, 4 * h:4 * h + 4, :],
                )
                all_gather(s2h_d[h], s2g_d[h])

            # ================= layer 2 out =================
            # out_st reuses a "wt" slot; allocated lazily AFTER o2's last wT
            # tile so the ring rotation never makes a wT load wait on the
            # final output stores.
            _oh = {}

            def out_drain(ch, po):
                if "t" not in _oh:
                    _oh["t"] = wtp.tile([128, 2, R], F32, tag="wt", name="out_st")
                out_st = _oh["t"]
                for mh in range(2):
                    nc.vector.tensor_copy(
                        out_st[:, ch, mh * 512:(mh + 1) * 512],
                        po[:, mh * 512:(mh + 1) * 512],
                    )
                    nc.scalar.dma_start(
                        out=outT.ap()[
                            ch * 128:(ch + 1) * 128, mh * 512:(mh + 1) * 512
                        ],
                        in_=out_st[:, ch, mh * 512:(mh + 1) * 512],
                    )

            o_phase(
                lambda qq, rk: s2g_d[qq // 2][
                    rk * 2 * Q + (qq % 2) * Q:rk * 2 * Q + (qq % 2 + 1) * Q, :
                ],
                out_drain,
                "o2",
            )

    nc.compile()
    return nc


_NC_CACHE = {}


def _get_nc():
    if "nc" not in _NC_CACHE:
        _NC_CACHE["nc"] = build_kernel()
    return _NC_CACHE["nc"]


# global pid-independent contraction order: quarter-major, rank-major
_PERM = np.concatenate(
    [
        np.arange(rk * R + q * Q, rk * R + q * Q + Q)
        for q in range(4)
        for rk in range(NCORES)
    ]
)


def make_in_maps(input, wavelets, wavelets_inv, W1, W2, filter1, filter2):
    input = np.asarray(input, np.float32)
    wavelets = np.asarray(wavelets, np.float32)
    wavelets_inv = np.asarray(wavelets_inv, np.float32)
    W1b = np.ascontiguousarray(np.asarray(W1, np.float32)).astype(NP_BF16)
    W2b = np.ascontiguousarray(np.asarray(W2, np.float32)).astype(NP_BF16)
    filter1 = np.asarray(filter1, np.float32)
    filter2 = np.asarray(filter2, np.float32)

    xT_pi = input.T[:, _PERM].astype(NP_BF16)
    # granule-major [32*F, Q]: granule gi contiguous; identical on every core
    xT_g = np.ascontiguousarray(
        np.concatenate([xT_pi[:, g * Q:(g + 1) * Q] for g in range(32)], axis=0)
    )
    in_maps = []
    for i in range(NCORES):
        r0, r1 = i * R, (i + 1) * R
        wvT_pi = wavelets_inv[r0:r1].T[_PERM].astype(NP_BF16)
        winvT_i = np.ascontiguousarray(
            np.concatenate(
                [wvT_pi[:, q * Q:(q + 1) * Q] for q in range(4)], axis=0
            )
        )  # q-major [4*N, Q]
        wT_i = np.ascontiguousarray(wavelets[r0:r1].T[_PERM]).astype(NP_BF16)
        in_maps.append(
            {
                "xT": xT_g,
                "w1": W1b,
                "w2": W2b,
                "winvT": winvT_i,
                "wT": wT_i,
                "f1": np.ascontiguousarray(filter1[r0:r1]),
                "f2": np.ascontiguousarray(filter2[r0:r1]),
            }
        )
    return in_maps


def run(in_maps, trace=False, **kw):
    nc = _get_nc()
    return bass_utils.run_bass_kernel_spmd(
        nc, in_maps, core_ids=list(range(NCORES)), trace=trace, **kw
    )


def kernel(input, wavelets, wavelets_inv, W1, W2, filter1, filter2):
    in_maps = make_in_maps(
        input, wavelets, wavelets_inv, W1, W2, filter1, filter2
    )
    res = run(in_maps)
    out = np.empty((N, C), np.float32)
    for i in range(NCORES):
        out[i * R:(i + 1) * R, :] = res.results[i]["outT"].T
    return out


# revision 13
# speedup vs baseline: 1.1554x; 1.0772x over previous
"""Graph Wavelet NN (2-layer) Trainium2 kernel, 8-core row-parallel, v5.

Math per layer: out = (wavelets * f) @ (wavelets_inv @ (x @ W)); the filter is
folded into a row-scale of the small spectral tensor s.

Design (measured: v1 477us, v2 550us, v3 607us, v4 566us — each traced):
- t1 = x @ W1 computed FULLY REPLICATED per core (27us of PE work, no
  cross-core deps): layer 1 has no input exchange, and a dummy AllGather at
  t=0 absorbs the one-time ~47us collective-stream init + launch skew.
- winvT SBUF-RESIDENT (16MB, filled during t1/s1, reused by s2); wT streamed
  twice (o1, o2): no phase window stacks two 16MB HBM streams.
- ONE total order sigma = (half, rank) of the 64 contraction blocks is baked
  into the host layouts of xT, winvT, wT.  Every DMA is a fully-contiguous
  block read (v3/v4 traces showed 512B-segmented and per-DMA fixed costs
  pacing t1 and the o-phases), and every stream is consumed strictly in load
  order.
- All exchanges are HALVES (256KB in / 2MB out per AllGather): s1 stores+AGs
  each half as its two column-quarter passes finish (pipelines E2 under s1);
  t2/s2 halves AG'd immediately.  o1/s2/o2 are slot-pipelined single passes
  consuming gathered rank-halves (256KB contiguous reads) in arrival order.
- Collectives alone on gpsimd; xq/winv/wT streams on sync; small loads,
  stores and gather reads on scalar.  bf16 matmuls, fp32 PSUM.
"""

import sys

if "/opt/trn_rl_repo" not in sys.path:
    sys.path.insert(0, "/opt/trn_rl_repo")

import numpy as np
import ml_dtypes

import concourse.bass as bass
import concourse.mybir as mybir
import concourse.tile as tile
from concourse import bacc, bass_utils

N = 8192
F = 512
C = 256
NCORES = 8
R = N // NCORES          # 1024 rows per core
H = R // 2               # 512-row half (exchange granule)
Q = R // 4               # 256-col quarter (s-phase output passes)
NSLOT = N // 128         # 64 contraction slots of 128 rows

F32 = mybir.dt.float32
BF16 = mybir.dt.bfloat16
NP_BF16 = ml_dtypes.bfloat16


def build_kernel(sim_single_core=False):
    nc = bacc.Bacc(
        "TRN2",
        target_bir_lowering=False,
        debug=False,
        num_devices=1 if sim_single_core else NCORES,
    )

    xT = nc.dram_tensor("xT", [16 * F, H], BF16, kind="ExternalInput")
    w1 = nc.dram_tensor("w1", [F, C], BF16, kind="ExternalInput")
    w2 = nc.dram_tensor("w2", [C, C], BF16, kind="ExternalInput")
    winvT = nc.dram_tensor("winvT", [4 * N, Q], BF16, kind="ExternalInput")
    wT = nc.dram_tensor("wT", [N, R], BF16, kind="ExternalInput")
    f1 = nc.dram_tensor("f1", [R], F32, kind="ExternalInput")
    f2 = nc.dram_tensor("f2", [R], F32, kind="ExternalInput")
    outT = nc.dram_tensor("outT", [C, R], F32, kind="ExternalOutput")

    rg = [list(range(NCORES))]

    with tile.TileContext(nc) as tc:
        with (
            tc.tile_pool(name="dram", bufs=1, space="DRAM") as dram,
            tc.tile_pool(name="const", bufs=1) as const,
            tc.tile_pool(name="stgp", bufs=2) as stgp,
            tc.tile_pool(name="wtp", bufs=2) as wtp,
            tc.tile_pool(name="xqp", bufs=2) as xqp,
            tc.tile_pool(name="tsp", bufs=4) as tsp,
            tc.tile_pool(name="psq", bufs=2, space="PSUM") as psq,
            tc.tile_pool(name="psO", bufs=2, space="PSUM") as psO,
        ):
            # ---- DRAM exchange buffers (halves) ----
            def mk_pair(nm):
                ins, outs = [], []
                for b in range(2):
                    ins.append(dram.tile([H, C], BF16, name=f"{nm}{b}_d"))
                    outs.append(
                        dram.tile(
                            [NCORES * H, C], BF16,
                            addr_space="Local" if sim_single_core else "Shared",
                            name=f"{nm}{b}g_d",
                        )
                    )
                return ins, outs

            s1h_d, s1g_d = mk_pair("s1")
            t2h_d, t2g_d = mk_pair("t2")
            s2h_d, s2g_d = mk_pair("s2")

            # dummy collective: starts the ncfw stream init at t~0 so the
            # first real AllGather doesn't eat the one-time cost.
            dum_i = dram.tile([Q, 1], BF16, name="dum_i")
            dum_o = dram.tile(
                [NCORES * Q, 1], BF16,
                addr_space="Local" if sim_single_core else "Shared",
                name="dum_o",
            )

            # ---- persistent SBUF ----
            winv_sb = const.tile([128, 4, NSLOT, Q], BF16)  # 128KB/part
            t1f_sb = const.tile([128, NSLOT, C], BF16)      # full t1, sigma
            w1_sb = const.tile([128, F // 128, C], BF16)
            w2_sb = const.tile([128, C // 128, C], BF16)
            f1_sb = const.tile([128, 8], F32)
            f2_sb = const.tile([128, 8], F32)

            def all_gather(in_d, out_d):
                if sim_single_core:
                    rows = in_d.shape[0]
                    for rr in range(NCORES):
                        nc.sync.dma_start(
                            out=out_d[rr * rows:(rr + 1) * rows, :], in_=in_d[:, :]
                        )
                else:
                    nc.gpsimd.collective_compute(
                        "AllGather",
                        mybir.AluOpType.bypass,
                        replica_groups=rg,
                        ins=[in_d.opt()],
                        outs=[out_d.opt()],
                    )

            all_gather(dum_i, dum_o)

            nc.scalar.dma_start(
                out=w1_sb[:], in_=w1.ap().rearrange("(kc p) n -> p kc n", p=128)
            )
            nc.scalar.dma_start(
                out=w2_sb[:], in_=w2.ap().rearrange("(kc p) n -> p kc n", p=128)
            )
            nc.scalar.dma_start(
                out=f1_sb[:], in_=f1.ap().rearrange("(mt p) -> p mt", p=128)
            )
            nc.scalar.dma_start(
                out=f2_sb[:], in_=f2.ap().rearrange("(mt p) -> p mt", p=128)
            )

            # ======= t1 = x @ W1, fully replicated, staged in sigma order =====
            # xq granule gi = 512 sigma-columns (4 slots), contiguous 512KB.
            for gi in range(16):
                xq = xqp.tile([128, 4, H], BF16, tag="xq", name=f"xq{gi}")
                nc.sync.dma_start(
                    out=xq[:],
                    in_=xT.ap()[gi * F:(gi + 1) * F, :].rearrange(
                        "(kc p) m -> p kc m", p=128
                    ),
                )
                for hf in range(2):
                    pt = psq.tile(
                        [128, 2, C], F32, tag="ps", name=f"pt1_{gi}_{hf}"
                    )
                    for j in range(2):
                        jj = 2 * hf + j
                        for kc in range(4):
                            nc.tensor.matmul(
                                pt[:, j, :],
                                xq[:, kc, jj * 128:(jj + 1) * 128],
                                w1_sb[:, kc, :],
                                start=(j == 0 and kc == 0),
                                stop=(kc == 3),
                                skip_group_check=True,
                            )
                        nc.vector.tensor_copy(
                            t1f_sb[:, 4 * gi + jj, :], pt[:, j, :]
                        )

            # winv fill: col-quarter-major (q, g) pieces, each contiguous.
            for q in range(4):
                for g in range(8):
                    nc.sync.dma_start(
                        out=winv_sb[:, q, g * 8:(g + 1) * 8, :],
                        in_=winvT.ap()[
                            q * N + g * 1024:q * N + (g + 1) * 1024, :
                        ].rearrange("(kc p) m -> p kc m", p=128),
                    )

            # ======= s1 = Winv @ t1 (all SBUF), col-quarter passes; AG halves =
            s_sb1 = stgp.tile([128, 8, C], BF16, tag="stg", name="s_sb1")
            for q in range(4):
                ps = psq.tile([128, 2, C], F32, tag="ps", name=f"ps1_{q}")
                for p in range(NSLOT):
                    for j in range(2):
                        nc.tensor.matmul(
                            ps[:, j, :],
                            winv_sb[:, q, p, j * 128:(j + 1) * 128],
                            t1f_sb[:, p, :],
                            start=(p == 0 and j == 0),
                            stop=(p == NSLOT - 1),
                            skip_group_check=True,
                        )
                for j in range(2):
                    nc.vector.tensor_scalar_mul(
                        s_sb1[:, 2 * q + j, :],
                        ps[:, j, :],
                        f1_sb[:, 2 * q + j:2 * q + j + 1],
                    )
                if q % 2 == 1:
                    h = q // 2
                    nc.scalar.dma_start(
                        out=s1h_d[h][:, :].rearrange("(k p) n -> p k n", p=128),
                        in_=s_sb1[:, 4 * h:4 * h + 4, :],
                    )
                    all_gather(s1h_d[h], s1g_d[h])

            # ---- o phase: out_loc = (w[rows]*f) @ s_full, slot-pipelined ----
            # consumes gathered rank-halves (256KB contiguous) in sigma order.
            def o_phase(sg_d, drain_cb, name):
                po = [
                    psO.tile([128, R], F32, tag="po", name=f"po_{name}{ch}")
                    for ch in range(2)
                ]
                wt_tiles = {}

                def load_wt(g):
                    t = wtp.tile([128, 4, R], BF16, tag="wt", name=f"wt_{name}{g}")
                    nc.sync.dma_start(
                        out=t[:],
                        in_=wT.ap()[g * 512:(g + 1) * 512, :].rearrange(
                            "(kc p) m -> p kc m", p=128
                        ),
                    )
                    wt_tiles[g] = t

                load_wt(0)
                load_wt(1)
                for h in range(2):
                    for rk in range(NCORES):
                        g = h * 8 + rk
                        sgt = tsp.tile(
                            [128, 4, C], BF16, tag="ts", name=f"so_{name}_{g}"
                        )
                        nc.scalar.dma_start(
                            out=sgt[:],
                            in_=sg_d[h][rk * H:(rk + 1) * H, :].rearrange(
                                "(k p) n -> p k n", p=128
                            ),
                        )
                        if g + 2 < 16:
                            load_wt(g + 2)
                        for jj in range(4):
                            c = 4 * g + jj
                            for ch in range(2):
                                for mh in range(2):
                                    nc.tensor.matmul(
                                        po[ch][:, mh * 512:(mh + 1) * 512],
                                        sgt[:, jj, ch * 128:(ch + 1) * 128],
                                        wt_tiles[g][
                                            :, jj, mh * 512:(mh + 1) * 512
                                        ],
                                        start=(c == 0),
                                        stop=(c == NSLOT - 1),
                                        skip_group_check=True,
                                    )
                for ch in range(2):
                    drain_cb(ch, po[ch])

            # ================= layer 1 out =================
            _h1 = {}

            def relu_drain(ch, po):
                if "t" not in _h1:
                    _h1["t"] = stgp.tile(
                        [128, C // 128, R], BF16, tag="stg", name="h1T_sb"
                    )
                h1T_sb = _h1["t"]
                for mh in range(2):
                    nc.vector.tensor_scalar_max(
                        h1T_sb[:, ch, mh * 512:(mh + 1) * 512],
                        po[:, mh * 512:(mh + 1) * 512],
                        0.0,
                    )

            o_phase(s1g_d, relu_drain, "o1")
            h1T_sb = _h1["t"]

            # ======= t2 = relu(o1) @ W2 (local rows), AG per half =======
            t_sb2 = stgp.tile([128, 8, C], BF16, tag="stg", name="t_sb2")
            for h in range(2):
                for q2 in range(2):
                    q = 2 * h + q2
                    pt = psq.tile([128, 2, C], F32, tag="ps", name=f"pt2_{q}")
                    for j in range(2):
                        mt = 2 * q + j
                        for kc in range(2):
                            nc.tensor.matmul(
                                pt[:, j, :],
                                h1T_sb[:, kc, mt * 128:(mt + 1) * 128],
                                w2_sb[:, kc, :],
                                start=(j == 0 and kc == 0),
                                stop=(kc == 1),
                                skip_group_check=True,
                            )
                        nc.vector.tensor_copy(t_sb2[:, mt, :], pt[:, j, :])
                nc.scalar.dma_start(
                    out=t2h_d[h][:, :].rearrange("(k p) n -> p k n", p=128),
                    in_=t_sb2[:, 4 * h:4 * h + 4, :],
                )
                all_gather(t2h_d[h], t2g_d[h])

            # ======= s2 = Winv @ t2_full, slot-pipelined single pass =======
            ps2 = [
                psq.tile([128, 2, 2, C], F32, tag="ps", name=f"ps2_{i}")
                for i in range(2)
            ]
            for h in range(2):
                for rk in range(NCORES):
                    g = h * 8 + rk
                    tsg = tsp.tile([128, 4, C], BF16, tag="ts", name=f"ts2_{g}")
                    nc.scalar.dma_start(
                        out=tsg[:],
                        in_=t2g_d[h][rk * H:(rk + 1) * H, :].rearrange(
                            "(k p) n -> p k n", p=128
                        ),
                    )
                    for jj in range(4):
                        c = 4 * g + jj
                        for mt in range(8):
                            nc.tensor.matmul(
                                ps2[mt // 4][:, (mt % 4) // 2, (mt % 4) % 2, :],
                                winv_sb[
                                    :, mt // 2, c,
                                    (mt % 2) * 128:(mt % 2 + 1) * 128,
                                ],
                                tsg[:, jj, :],
                                start=(c == 0 and mt % 2 == 0),
                                stop=(c == NSLOT - 1),
                                skip_group_check=True,
                            )

            s_sb2 = stgp.tile([128, 8, C], BF16, tag="stg", name="s_sb2")
            for h in range(2):
                for j in range(4):
                    mt = 4 * h + j
                    nc.vector.tensor_scalar_mul(
                        s_sb2[:, mt, :],
                        ps2[mt // 4][:, (mt % 4) // 2, (mt % 4) % 2, :],
                        f2_sb[:, mt:mt + 1],
                    )
                nc.scalar.dma_start(
                    out=s2h_d[h][:, :].rearrange("(k p) n -> p k n", p=128),
                    in_=s_sb2[:, 4 * h:4 * h + 4, :],
                )
                all_gather(s2h_d[h], s2g_d[h])

            # ================= layer 2 out =================
            # out_st reuses a "wt" slot; allocated lazily AFTER o2's last wT
            # tile so the ring rotation never makes a wT load wait on the
            # final output stores.
            _oh = {}

            def out_drain(ch, po):
                if "t" not in _oh:
                    _oh["t"] = wtp.tile([128, 2, R], F32, tag="wt", name="out_st")
                out_st = _oh["t"]
                for mh in range(2):
                    nc.vector.tensor_copy(
                        out_st[:, ch, mh * 512:(mh + 1) * 512],
                        po[:, mh * 512:(mh + 1) * 512],
                    )
                    nc.scalar.dma_start(
                        out=outT.ap()[
                            ch * 128:(ch + 1) * 128, mh * 512:(mh + 1) * 512
                        ],
                        in_=out_st[:, ch, mh * 512:(mh + 1) * 512],
                    )

            o_phase(s2g_d, out_drain, "o2")

    nc.compile()
    return nc


_NC_CACHE = {}


def _get_nc():
    if "nc" not in _NC_CACHE:
        _NC_CACHE["nc"] = build_kernel()
    return _NC_CACHE["nc"]


# global sigma order: half-major, rank-major 512-row blocks
_PERM = np.concatenate(
    [
        np.arange(rk * R + h * H, rk * R + h * H + H)
        for h in range(2)
        for rk in range(NCORES)
    ]
)


def make_in_maps(input, wavelets, wavelets_inv, W1, W2, filter1, filter2):
    input = np.asarray(input, np.float32)
    wavelets = np.asarray(wavelets, np.float32)
    wavelets_inv = np.asarray(wavelets_inv, np.float32)
    W1b = np.ascontiguousarray(np.asarray(W1, np.float32)).astype(NP_BF16)
    W2b = np.ascontiguousarray(np.asarray(W2, np.float32)).astype(NP_BF16)
    filter1 = np.asarray(filter1, np.float32)
    filter2 = np.asarray(filter2, np.float32)

    xT_pi = input.T[:, _PERM].astype(NP_BF16)
    # granule-major [16*F, H]: granule gi contiguous; identical on every core
    xT_g = np.ascontiguousarray(
        np.concatenate([xT_pi[:, g * H:(g + 1) * H] for g in range(16)], axis=0)
    )
    in_maps = []
    for i in range(NCORES):
        r0, r1 = i * R, (i + 1) * R
        wvT_pi = wavelets_inv[r0:r1].T[_PERM].astype(NP_BF16)
        winvT_i = np.ascontiguousarray(
            np.concatenate(
                [wvT_pi[:, q * Q:(q + 1) * Q] for q in range(4)], axis=0
            )
        )  # col-quarter-major [4*N, Q]
        wT_i = np.ascontiguousarray(wavelets[r0:r1].T[_PERM]).astype(NP_BF16)
        in_maps.append(
            {
                "xT": xT_g,
                "w1": W1b,
                "w2": W2b,
                "winvT": winvT_i,
                "wT": wT_i,
                "f1": np.ascontiguousarray(filter1[r0:r1]),
                "f2": np.ascontiguousarray(filter2[r0:r1]),
            }
        )
    return in_maps


def run(in_maps, trace=False, **kw):
    nc = _get_nc()
    return bass_utils.run_bass_kernel_spmd(
        nc, in_maps, core_ids=list(range(NCORES)), trace=trace, **kw
    )


def kernel(input, wavelets, wavelets_inv, W1, W2, filter1, filter2):
    in_maps = make_in_maps(
        input, wavelets, wavelets_inv, W1, W2, filter1, filter2
    )
    res = run(in_maps)
    out = np.empty((N, C), np.float32)
    for i in range(NCORES):
        out[i * R:(i + 1) * R, :] = res.results[i]["outT"].T
    return out


# revision 14
# speedup vs baseline: 1.2843x; 1.1116x over previous
"""Graph Wavelet NN (2-layer) Trainium2 kernel, 8-core row-parallel, v5.

Math per layer: out = (wavelets * f) @ (wavelets_inv @ (x @ W)); the filter is
folded into a row-scale of the small spectral tensor s.

Design (measured: v1 477us, v2 550us, v3 607us, v4 566us — each traced):
- t1 = x @ W1 computed FULLY REPLICATED per core (27us of PE work, no
  cross-core deps): layer 1 has no input exchange, and a dummy AllGather at
  t=0 absorbs the one-time ~47us collective-stream init + launch skew.
- winvT SBUF-RESIDENT (16MB, filled during t1/s1, reused by s2); wT streamed
  twice (o1, o2): no phase window stacks two 16MB HBM streams.
- ONE total order sigma = (half, rank) of the 64 contraction blocks is baked
  into the host layouts of xT, winvT, wT.  Every DMA is a fully-contiguous
  block read (v3/v4 traces showed 512B-segmented and per-DMA fixed costs
  pacing t1 and the o-phases), and every stream is consumed strictly in load
  order.
- All exchanges are HALVES (256KB in / 2MB out per AllGather): s1 stores+AGs
  each half as its two column-quarter passes finish (pipelines E2 under s1);
  t2/s2 halves AG'd immediately.  o1/s2/o2 are slot-pipelined single passes
  consuming gathered rank-halves (256KB contiguous reads) in arrival order.
- Collectives alone on gpsimd; xq/winv/wT streams on sync; small loads,
  stores and gather reads on scalar.  bf16 matmuls, fp32 PSUM.
"""

import sys

if "/opt/trn_rl_repo" not in sys.path:
    sys.path.insert(0, "/opt/trn_rl_repo")

import numpy as np
import ml_dtypes

import concourse.bass as bass
import concourse.mybir as mybir
import concourse.tile as tile
from concourse import bacc, bass_utils

N = 8192
F = 512
C = 256
NCORES = 8
R = N // NCORES          # 1024 rows per core
H = R // 2               # 512-row half (exchange granule)
Q = R // 4               # 256-col quarter (s-phase output passes)
NSLOT = N // 128         # 64 contraction slots of 128 rows

F32 = mybir.dt.float32
BF16 = mybir.dt.bfloat16
NP_BF16 = ml_dtypes.bfloat16


def build_kernel(sim_single_core=False):
    nc = bacc.Bacc(
        "TRN2",
        target_bir_lowering=False,
        debug=False,
        num_devices=1 if sim_single_core else NCORES,
    )

    xT = nc.dram_tensor("xT", [16 * F, H], BF16, kind="ExternalInput")
    w1 = nc.dram_tensor("w1", [F, C], BF16, kind="ExternalInput")
    w2 = nc.dram_tensor("w2", [C, C], BF16, kind="ExternalInput")
    winvT = nc.dram_tensor("winvT", [4 * N, Q], BF16, kind="ExternalInput")
    wT = nc.dram_tensor("wT", [N, R], BF16, kind="ExternalInput")
    f1 = nc.dram_tensor("f1", [R], F32, kind="ExternalInput")
    f2 = nc.dram_tensor("f2", [R], F32, kind="ExternalInput")
    outT = nc.dram_tensor("outT", [C, R], F32, kind="ExternalOutput")

    rg = [list(range(NCORES))]

    with tile.TileContext(nc) as tc:
        with (
            tc.tile_pool(name="dram", bufs=1, space="DRAM") as dram,
            tc.tile_pool(name="const", bufs=1) as const,
            tc.tile_pool(name="stgp", bufs=2) as stgp,
            tc.tile_pool(name="wtp", bufs=3) as wtp,
            tc.tile_pool(name="tsp", bufs=3) as tsp,
            tc.tile_pool(name="psq", bufs=2, space="PSUM") as psq,
            tc.tile_pool(name="psO", bufs=2, space="PSUM") as psO,
        ):
            # ---- DRAM exchange buffers (halves) ----
            def mk_pair(nm):
                ins, outs = [], []
                for b in range(2):
                    ins.append(dram.tile([H, C], BF16, name=f"{nm}{b}_d"))
                    outs.append(
                        dram.tile(
                            [NCORES * H, C], BF16,
                            addr_space="Local" if sim_single_core else "Shared",
                            name=f"{nm}{b}g_d",
                        )
                    )
                return ins, outs

            s1h_d, s1g_d = mk_pair("s1")
            t2h_d, t2g_d = mk_pair("t2")
            s2h_d, s2g_d = mk_pair("s2")

            # dummy collective: starts the ncfw stream init at t~0 so the
            # first real AllGather doesn't eat the one-time cost.
            dum_i = dram.tile([Q, 1], BF16, name="dum_i")
            dum_o = dram.tile(
                [NCORES * Q, 1], BF16,
                addr_space="Local" if sim_single_core else "Shared",
                name="dum_o",
            )

            # ---- persistent SBUF ----
            winv_sb = const.tile([128, 4, NSLOT, Q], BF16)  # 128KB/part
            t1f_sb = const.tile([128, NSLOT, C], BF16)      # full t1, sigma
            w1_sb = const.tile([128, F // 128, C], BF16)
            w2_sb = const.tile([128, C // 128, C], BF16)
            f1_sb = const.tile([128, 8], F32)
            f2_sb = const.tile([128, 8], F32)

            def all_gather(in_d, out_d):
                if sim_single_core:
                    rows = in_d.shape[0]
                    for rr in range(NCORES):
                        nc.sync.dma_start(
                            out=out_d[rr * rows:(rr + 1) * rows, :], in_=in_d[:, :]
                        )
                else:
                    nc.gpsimd.collective_compute(
                        "AllGather",
                        mybir.AluOpType.bypass,
                        replica_groups=rg,
                        ins=[in_d.opt()],
                        outs=[out_d.opt()],
                    )

            all_gather(dum_i, dum_o)

            nc.scalar.dma_start(
                out=w1_sb[:], in_=w1.ap().rearrange("(kc p) n -> p kc n", p=128)
            )
            nc.scalar.dma_start(
                out=w2_sb[:], in_=w2.ap().rearrange("(kc p) n -> p kc n", p=128)
            )
            nc.scalar.dma_start(
                out=f1_sb[:], in_=f1.ap().rearrange("(mt p) -> p mt", p=128)
            )
            nc.scalar.dma_start(
                out=f2_sb[:], in_=f2.ap().rearrange("(mt p) -> p mt", p=128)
            )

            # ======= t1 = x @ W1, fully replicated, staged in sigma order =====
            # xq granule gi = 512 sigma-columns (4 slots), contiguous 512KB.
            for gi in range(16):
                xq = tsp.tile([128, 4, H], BF16, tag="ts", name=f"xq{gi}")
                nc.sync.dma_start(
                    out=xq[:],
                    in_=xT.ap()[gi * F:(gi + 1) * F, :].rearrange(
                        "(kc p) m -> p kc m", p=128
                    ),
                )
                for hf in range(2):
                    pt = psq.tile(
                        [128, 2, C], F32, tag="ps", name=f"pt1_{gi}_{hf}"
                    )
                    for j in range(2):
                        jj = 2 * hf + j
                        for kc in range(4):
                            nc.tensor.matmul(
                                pt[:, j, :],
                                xq[:, kc, jj * 128:(jj + 1) * 128],
                                w1_sb[:, kc, :],
                                start=(j == 0 and kc == 0),
                                stop=(kc == 3),
                                skip_group_check=True,
                            )
                        nc.vector.tensor_copy(
                            t1f_sb[:, 4 * gi + jj, :], pt[:, j, :]
                        )

            # winv fill: col-quarter-major 2MB pieces, each contiguous.
            for q in range(4):
                for g2 in range(2):
                    nc.sync.dma_start(
                        out=winv_sb[:, q, g2 * 32:(g2 + 1) * 32, :],
                        in_=winvT.ap()[
                            q * N + g2 * 4096:q * N + (g2 + 1) * 4096, :
                        ].rearrange("(kc p) m -> p kc m", p=128),
                    )

            # ======= s1 = Winv @ t1 (all SBUF), col-quarter passes; AG halves =
            s_sb1 = stgp.tile([128, 8, C], BF16, tag="stg", name="s_sb1")
            for q in range(4):
                ps = psq.tile([128, 2, C], F32, tag="ps", name=f"ps1_{q}")
                for p in range(NSLOT):
                    for j in range(2):
                        nc.tensor.matmul(
                            ps[:, j, :],
                            winv_sb[:, q, p, j * 128:(j + 1) * 128],
                            t1f_sb[:, p, :],
                            start=(p == 0 and j == 0),
                            stop=(p == NSLOT - 1),
                            skip_group_check=True,
                        )
                for j in range(2):
                    nc.vector.tensor_scalar_mul(
                        s_sb1[:, 2 * q + j, :],
                        ps[:, j, :],
                        f1_sb[:, 2 * q + j:2 * q + j + 1],
                    )
                if q % 2 == 1:
                    h = q // 2
                    nc.scalar.dma_start(
                        out=s1h_d[h][:, :].rearrange("(k p) n -> p k n", p=128),
                        in_=s_sb1[:, 4 * h:4 * h + 4, :],
                    )
                    all_gather(s1h_d[h], s1g_d[h])

            # ---- o phase: out_loc = (w[rows]*f) @ s_full, slot-pipelined ----
            # consumes gathered rank-halves (256KB contiguous) in sigma order.
            def o_phase(sg_d, drain_cb, name):
                po = [
                    psO.tile([128, R], F32, tag="po", name=f"po_{name}{ch}")
                    for ch in range(2)
                ]
                wt_tiles = {}

                def load_wt(g):
                    t = wtp.tile([128, 4, R], BF16, tag="wt", name=f"wt_{name}{g}")
                    nc.sync.dma_start(
                        out=t[:],
                        in_=wT.ap()[g * 512:(g + 1) * 512, :].rearrange(
                            "(kc p) m -> p kc m", p=128
                        ),
                    )
                    wt_tiles[g] = t

                load_wt(0)
                load_wt(1)
                for h in range(2):
                    for rk in range(NCORES):
                        g = h * 8 + rk
                        sgt = tsp.tile(
                            [128, 4, C], BF16, tag="ts", name=f"so_{name}_{g}"
                        )
                        nc.scalar.dma_start(
                            out=sgt[:],
                            in_=sg_d[h][rk * H:(rk + 1) * H, :].rearrange(
                                "(k p) n -> p k n", p=128
                            ),
                        )
                        if g + 2 < 16:
                            load_wt(g + 2)
                        for jj in range(4):
                            c = 4 * g + jj
                            for ch in range(2):
                                for mh in range(2):
                                    nc.tensor.matmul(
                                        po[ch][:, mh * 512:(mh + 1) * 512],
                                        sgt[:, jj, ch * 128:(ch + 1) * 128],
                                        wt_tiles[g][
                                            :, jj, mh * 512:(mh + 1) * 512
                                        ],
                                        start=(c == 0),
                                        stop=(c == NSLOT - 1),
                                        skip_group_check=True,
                                    )
                for ch in range(2):
                    drain_cb(ch, po[ch])

            # ================= layer 1 out =================
            _h1 = {}

            def relu_drain(ch, po):
                if "t" not in _h1:
                    _h1["t"] = stgp.tile(
                        [128, C // 128, R], BF16, tag="stg", name="h1T_sb"
                    )
                h1T_sb = _h1["t"]
                for mh in range(2):
                    nc.vector.tensor_scalar_max(
                        h1T_sb[:, ch, mh * 512:(mh + 1) * 512],
                        po[:, mh * 512:(mh + 1) * 512],
                        0.0,
                    )

            o_phase(s1g_d, relu_drain, "o1")
            h1T_sb = _h1["t"]

            # ======= t2 = relu(o1) @ W2 (local rows), AG per half =======
            t_sb2 = stgp.tile([128, 8, C], BF16, tag="stg", name="t_sb2")
            for h in range(2):
                for q2 in range(2):
                    q = 2 * h + q2
                    pt = psq.tile([128, 2, C], F32, tag="ps", name=f"pt2_{q}")
                    for j in range(2):
                        mt = 2 * q + j
                        for kc in range(2):
                            nc.tensor.matmul(
                                pt[:, j, :],
                                h1T_sb[:, kc, mt * 128:(mt + 1) * 128],
                                w2_sb[:, kc, :],
                                start=(j == 0 and kc == 0),
                                stop=(kc == 1),
                                skip_group_check=True,
                            )
                        nc.vector.tensor_copy(t_sb2[:, mt, :], pt[:, j, :])
                nc.scalar.dma_start(
                    out=t2h_d[h][:, :].rearrange("(k p) n -> p k n", p=128),
                    in_=t_sb2[:, 4 * h:4 * h + 4, :],
                )
                all_gather(t2h_d[h], t2g_d[h])

            # ======= s2 = Winv @ t2_full, slot-pipelined single pass =======
            ps2 = [
                psq.tile([128, 2, 2, C], F32, tag="ps", name=f"ps2_{i}")
                for i in range(2)
            ]
            for h in range(2):
                for rk in range(NCORES):
                    g = h * 8 + rk
                    tsg = tsp.tile([128, 4, C], BF16, tag="ts", name=f"ts2_{g}")
                    nc.scalar.dma_start(
                        out=tsg[:],
                        in_=t2g_d[h][rk * H:(rk + 1) * H, :].rearrange(
                            "(k p) n -> p k n", p=128
                        ),
                    )
                    for jj in range(4):
                        c = 4 * g + jj
                        for mt in range(8):
                            nc.tensor.matmul(
                                ps2[mt // 4][:, (mt % 4) // 2, (mt % 4) % 2, :],
                                winv_sb[
                                    :, mt // 2, c,
                                    (mt % 2) * 128:(mt % 2 + 1) * 128,
                                ],
                                tsg[:, jj, :],
                                start=(c == 0 and mt % 2 == 0),
                                stop=(c == NSLOT - 1),
                                skip_group_check=True,
                            )

            s_sb2 = stgp.tile([128, 8, C], BF16, tag="stg", name="s_sb2")
            for h in range(2):
                for j in range(4):
                    mt = 4 * h + j
                    nc.vector.tensor_scalar_mul(
                        s_sb2[:, mt, :],
                        ps2[mt // 4][:, (mt % 4) // 2, (mt % 4) % 2, :],
                        f2_sb[:, mt:mt + 1],
                    )
                nc.scalar.dma_start(
                    out=s2h_d[h][:, :].rearrange("(k p) n -> p k n", p=128),
                    in_=s_sb2[:, 4 * h:4 * h + 4, :],
                )
                all_gather(s2h_d[h], s2g_d[h])

            # ================= layer 2 out =================
            # out_st reuses a "wt" slot; allocated lazily AFTER o2's last wT
            # tile so the ring rotation never makes a wT load wait on the
            # final output stores.
            _oh = {}

            def out_drain(ch, po):
                if "t" not in _oh:
                    _oh["t"] = wtp.tile([128, 2, R], F32, tag="wt", name="out_st")
                out_st = _oh["t"]
                for mh in range(2):
                    nc.vector.tensor_copy(
                        out_st[:, ch, mh * 512:(mh + 1) * 512],
                        po[:, mh * 512:(mh + 1) * 512],
                    )
                    nc.scalar.dma_start(
                        out=outT.ap()[
                            ch * 128:(ch + 1) * 128, mh * 512:(mh + 1) * 512
                        ],
                        in_=out_st[:, ch, mh * 512:(mh + 1) * 512],
                    )

            o_phase(s2g_d, out_drain, "o2")

    nc.compile()
    return nc


_NC_CACHE = {}


def _get_nc():
    if "nc" not in _NC_CACHE:
        _NC_CACHE["nc"] = build_kernel()
    return _NC_CACHE["nc"]


# global sigma order: half-major, rank-major 512-row blocks
_PERM = np.concatenate(
    [
        np.arange(rk * R + h * H, rk * R + h * H + H)
        for h in range(2)
        for rk in range(NCORES)
    ]
)


def make_in_maps(input, wavelets, wavelets_inv, W1, W2, filter1, filter2):
    input = np.asarray(input, np.float32)
    wavelets = np.asarray(wavelets, np.float32)
    wavelets_inv = np.asarray(wavelets_inv, np.float32)
    W1b = np.ascontiguousarray(np.asarray(W1, np.float32)).astype(NP_BF16)
    W2b = np.ascontiguousarray(np.asarray(W2, np.float32)).astype(NP_BF16)
    filter1 = np.asarray(filter1, np.float32)
    filter2 = np.asarray(filter2, np.float32)

    xT_pi = input.T[:, _PERM].astype(NP_BF16)
    # granule-major [16*F, H]: granule gi contiguous; identical on every core
    xT_g = np.ascontiguousarray(
        np.concatenate([xT_pi[:, g * H:(g + 1) * H] for g in range(16)], axis=0)
    )
    in_maps = []
    for i in range(NCORES):
        r0, r1 = i * R, (i + 1) * R
        wvT_pi = wavelets_inv[r0:r1].T[_PERM].astype(NP_BF16)
        winvT_i = np.ascontiguousarray(
            np.concatenate(
                [wvT_pi[:, q * Q:(q + 1) * Q] for q in range(4)], axis=0
            )
        )  # col-quarter-major [4*N, Q]
        wT_i = np.ascontiguousarray(wavelets[r0:r1].T[_PERM]).astype(NP_BF16)
        in_maps.append(
            {
                "xT": xT_g,
                "w1": W1b,
                "w2": W2b,
                "winvT": winvT_i,
                "wT": wT_i,
                "f1": np.ascontiguousarray(filter1[r0:r1]),
                "f2": np.ascontiguousarray(filter2[r0:r1]),
            }
        )
    return in_maps


def run(in_maps, trace=False, **kw):
    nc = _get_nc()
    return bass_utils.run_bass_kernel_spmd(
        nc, in_maps, core_ids=list(range(NCORES)), trace=trace, **kw
    )


def kernel(input, wavelets, wavelets_inv, W1, W2, filter1, filter2):
    in_maps = make_in_maps(
        input, wavelets, wavelets_inv, W1, W2, filter1, filter2
    )
    res = run(in_maps)
    out = np.empty((N, C), np.float32)
    for i in range(NCORES):
        out[i * R:(i + 1) * R, :] = res.results[i]["outT"].T
    return out
